# revision 81
# baseline (speedup 1.0000x reference)
"""Trainium2 Bass kernel for chess-structured multi-head attention (8 cores).

Math (per board b of 2048, S=64 squares, D=512, H=8 heads, HD=64):
  q/k/v = x @ W{q,k,v}.T + b{q,k,v}
  scores_h = q_h k_h^T / 8, masked per head (6 static chess relations,
  ray, attack), softmax over targets, out = concat_h(attn_h v_h) @ Wo.T + bo

Sharding: pure data parallel, 256 boards per core; weights replicated.

Key tricks vs the v1 kernel (1.007ms):
  - All four projections run as fp8e4m3 DoubleRow matmuls with hi+lo error
    compensation: x = xh + xl (both fp8), W*64 = Wh + Wl; three DR terms
    xh*Wh + xh*Wl + xl*Wh at 0.5 cyc/row gives 0.75x the rows of bf16 at
    bf16-level accuracy (validated offline: rel err 2.5e-3 vs ref).
  - softmax shift-invariance: bk dropped entirely (q.bk const per row);
    SCALE and bq folded into Wq/bq on host; bv exits the device via
    bo' = bo + bv @ Wo.T added on host.
  - scores: per (board, jj) ONE matmul computes both heads 2jj/2jj+1 via a
    block-diagonal stationary q2 = [[q_even,0],[0,q_odd]] (s-parity-major
    free layout), out spans all 128 partitions -> 2048 rows/g8.
  - masks applied MULTIPLICATIVELY post-exp (fp16 0/1 masks, DVE 2x mode).
  - attn@v: per (at-block, col-parity) ONE matmul computes two heads via
    block-diag V tiles vb2 -> 2048 rows/g8.
  - intermediates fp16 (not bf16): same speed, 8x less quantization noise.
  - output written fp16; host upcasts and adds bo'.
"""

import os
import sys
from contextlib import ExitStack

import numpy as np
import ml_dtypes

for _p in ("/opt/trn_rl_repo", os.path.expanduser("~/.axon_site/_ro/trn_rl_repo")):
    if os.path.isdir(_p) and _p not in sys.path:
        sys.path.append(_p)

import concourse.bass as bass
import concourse.tile as tile
from concourse import bacc, mybir
from concourse.bass_utils import run_bass_kernel_spmd

F8 = mybir.dt.float8e4
BF16 = mybir.dt.bfloat16
FP16 = mybir.dt.float16
F32 = mybir.dt.float32
f8 = ml_dtypes.float8_e4m3
bf16 = ml_dtypes.bfloat16
fp16 = np.float16

B, S, D, H, HD = 2048, 64, 512, 8, 64
NCORES = 8
BPC = B // NCORES
SCALE = float(1.0 / np.sqrt(HD))
WS = 64.0                       # weight pre-scale into fp8 normal range
DR = mybir.MatmulPerfMode.DoubleRow



def _static_masks():
    sq = np.arange(64)
    r = sq // 8
    f = sq % 8
    ri, rj = r[:, None], r[None, :]
    fi, fj = f[:, None], f[None, :]
    dr = np.abs(ri - rj)
    df = np.abs(fi - fj)
    eye = np.eye(64, dtype=bool)
    file_m = fi == fj
    rank_m = ri == rj
    diag_m = (ri - fi) == (rj - fj)
    adiag_m = (ri + fi) == (rj + fj)
    knight_m = (((dr == 2) & (df == 1)) | ((dr == 1) & (df == 2))) | eye
    king_m = (dr <= 1) & (df <= 1)
    return np.stack([file_m, rank_m, diag_m, adiag_m, knight_m, king_m])


def build_nc(n_boards=BPC):
    """Single-core Bass program (SPMD across 8 cores)."""
    assert n_boards % 8 == 0
    n_g8 = n_boards // 8
    TOK = n_boards * S

    nc = bacc.Bacc(None)

    # DRAM I/O.  x hi/lo fp8 chunks: [4 kchunks, 128, TOK]
    xh_d = nc.dram_tensor("xh", [4, 128, TOK], F8, kind="ExternalInput")
    xl_d = nc.dram_tensor("xl", [4, 128, TOK], F8, kind="ExternalInput")
    dynm_d = nc.dram_tensor("dynm", [n_g8, 128, 512], FP16,
                            kind="ExternalInput")
    w_d = {}
    for name in ("wqh", "wql", "wkh", "wkl", "wvh", "wvl", "woh", "wol"):
        w_d[name] = nc.dram_tensor(name, [4, 128, 512], F8,
                                   kind="ExternalInput")
    bq_d = nc.dram_tensor("bqs", [128, 4], F32, kind="ExternalInput")
    statm_d = nc.dram_tensor("statm", [128, 192], FP16, kind="ExternalInput")
    out = nc.dram_tensor("out", [TOK, 512], FP16, kind="ExternalOutput")

    AF = mybir.ActivationFunctionType
    ALU = mybir.AluOpType

    def chain(insts):
        for b_ in insts[1:]:
            tile.add_dep_helper(b_.ins, insts[0].ins, sync=False,
                                reason="psum group start-first")
        for a in insts[1:-1]:
            tile.add_dep_helper(insts[-1].ins, a.ins, sync=False,
                                reason="psum group stop-last")

    with tile.TileContext(nc) as tc, ExitStack() as ctx:
        const = ctx.enter_context(tc.tile_pool(name="const", bufs=1))
        p_x = ctx.enter_context(tc.tile_pool(name="x", bufs=3))
        p_qkv = ctx.enter_context(tc.tile_pool(name="qkv", bufs=3))
        p_blk = ctx.enter_context(tc.tile_pool(name="blk", bufs=4))
        p_sc = ctx.enter_context(tc.tile_pool(name="sc", bufs=4))
        p_at = ctx.enter_context(tc.tile_pool(name="at", bufs=4))
        p_ao = ctx.enter_context(tc.tile_pool(name="ao", bufs=3))
        p_out = ctx.enter_context(tc.tile_pool(name="out", bufs=3))
        # PSUM: 8 banks total.  proj 3 + scores 2 + (ao|oproj shared) 3
        ps_proj = ctx.enter_context(
            tc.tile_pool(name="ps_proj", bufs=3, space="PSUM"))
        ps_sc = ctx.enter_context(
            tc.tile_pool(name="ps_sc", bufs=2, space="PSUM"))
        ps_ao = ctx.enter_context(
            tc.tile_pool(name="ps_ao", bufs=3, space="PSUM"))
        ps_op = ps_ao

        # ---- constants (Q weights first, then iter-0/1 x prefetch,
        #      so the first projection can start ~6us earlier) ----
        w_sb = {}
        preload = {}
        # interleave: first Q matmul needs only wqh+xh0; wql+xl0 next
        w = const.tile([128, 4, 512], F8, tag="wqh", name="w")
        nc.sync.dma_start(out=w[:], in_=w_d["wqh"].rearrange("k p n -> p k n"))
        w_sb["wqh"] = w
        pxh = p_x.tile([128, 4, 512], F8, tag="xh", name="xh_t")
        nc.sync.dma_start(out=pxh[:], in_=xh_d[:, :, 0:512]
                          .rearrange("k p n -> p k n"))
        w = const.tile([128, 4, 512], F8, tag="wql", name="w")
        nc.sync.dma_start(out=w[:], in_=w_d["wql"].rearrange("k p n -> p k n"))
        w_sb["wql"] = w
        pxl = p_x.tile([128, 4, 512], F8, tag="xl", name="xl_t")
        nc.sync.dma_start(out=pxl[:], in_=xl_d[:, :, 0:512]
                          .rearrange("k p n -> p k n"))
        preload[0] = (pxh, pxl)
        bq_sb = const.tile([128, 4], F32, tag="bqs", name="bq_sb")
        nc.sync.dma_start(out=bq_sb[:], in_=bq_d[:])
        # dummy exp: hoists the one-time ACT table load into the DMA wait
        warm_t = const.tile([128, 1], F32, tag="warm", name="warm_t")
        nc.scalar.activation(warm_t[:], bq_sb[:, 0:1], AF.Exp, scale=0.0)
        for name, t in w_d.items():
            if name in w_sb:
                continue
            w = const.tile([128, 4, 512], F8, tag=name, name="w")
            nc.sync.dma_start(out=w[:], in_=t.rearrange("k p n -> p k n"))
            w_sb[name] = w
        stat_sb = const.tile([128, 192], FP16, tag="statm")
        nc.sync.dma_start(out=stat_sb[:], in_=statm_d[:])

        def proj_dr(psout, wh, wl, xh_t, xl_t, col0, terms=3):
            """Compensated fp8 DR projection into psout [128, 512].

            terms=3: xh*Wh + xh*Wl + xl*Wh;  terms=2 drops xl*Wh (used for
            Q only: softmax tolerates ~1e-2 there, halves nothing else).
            lhsT = w[:, kp-pair, col0:col0+128]; rhs = x[:, kp-pair, :].
            """
            half = terms == 2.5
            tl = ((wh, xh_t), (wl, xh_t), (wh, xl_t))[:2 if half else terms]
            n = 2 * len(tl) + (1 if half else 0)
            mms = []
            for kp in range(2):
                sl = (slice(None), slice(2 * kp, 2 * kp + 2),
                      slice(col0, col0 + 128))
                for wt, xt_ in tl:
                    mms.append(nc.tensor.matmul(
                        psout[:], wt[sl], xt_[:, 2 * kp:2 * kp + 2, :],
                        start=(len(mms) == 0), stop=(len(mms) == n - 1),
                        perf_mode=DR))
                if half and kp == 0:
                    mms.append(nc.tensor.matmul(
                        psout[:], wh[sl], xl_t[:, 0:2, :],
                        start=False, stop=False, perf_mode=DR))
            chain(mms)

        state = {}

        def front(g):
            """Loads, Q/K/V projections, scores, softmax -> state[g]."""
            tok0 = g * 512

            # ---- loads ----
            if g in preload:
                xh_t, xl_t = preload.pop(g)
            else:
                xh_t = p_x.tile([128, 4, 512], F8, tag="xh", name="xh_t")
                xl_t = p_x.tile([128, 4, 512], F8, tag="xl", name="xl_t")
                nc.sync.dma_start(out=xh_t[:],
                                  in_=xh_d[:, :, tok0:tok0 + 512]
                                  .rearrange("k p n -> p k n"))
                nc.sync.dma_start(out=xl_t[:],
                                  in_=xl_d[:, :, tok0:tok0 + 512]
                                  .rearrange("k p n -> p k n"))
            dyn_t = p_sc.tile([128, 512], FP16, tag="dyn", name="dyn_t")
            nc.sync.dma_start(out=dyn_t[:], in_=dynm_d[g, :, :])

            # ---- Q/K projections (transposed: d on partitions) ----
            qt_t = p_qkv.tile([128, 4, 512], FP16, tag="qt", name="qt_t")
            kt_t = p_qkv.tile([128, 4, 512], FP16, tag="kt", name="kt_t")
            for jj in range(4):
                ps = ps_proj.tile([128, 512], F32, tag="proj")
                proj_dr(ps, w_sb["wqh"], w_sb["wql"], xh_t, xl_t,
                        128 * jj, terms=2)
                nc.scalar.activation(qt_t[:, jj, :], ps[:], AF.Identity,
                                     bias=bq_sb[:, jj:jj + 1], scale=1.0 / WS)
            for jj in range(4):
                ps = ps_proj.tile([128, 512], F32, tag="proj")
                proj_dr(ps, w_sb["wkh"], w_sb["wkl"], xh_t, xl_t,
                        128 * jj, terms=2.5)
                nc.scalar.activation(kt_t[:, jj, :], ps[:], AF.Identity,
                                     scale=1.0 / WS)

            # ---- V projection (natural: tokens on partitions) ----
            v_t = p_qkv.tile([128, 4, 512], FP16, tag="v", name="v_t")
            for mt in range(4):
                ps = ps_proj.tile([128, 512], F32, tag="proj")
                mms = []
                for kp in range(2):
                    xsl = (slice(None), slice(2 * kp, 2 * kp + 2),
                           slice(128 * mt, 128 * mt + 128))
                    for wt, xt_ in (("wvh", xh_t), ("wvl", xh_t),
                                    ("wvh", xl_t)):
                        mms.append(nc.tensor.matmul(
                            ps[:], xt_[xsl],
                            w_sb[wt][:, 2 * kp:2 * kp + 2, :],
                            start=(len(mms) == 0), stop=(len(mms) == 5),
                            perf_mode=DR))
                chain(mms)
                nc.vector.tensor_scalar_mul(v_t[:, mt, :], ps[:], 1.0 / WS)

            # vv: zero-padded V tiles: vv[rh][b01] has board-b01 tokens
            # live on rows 64*rh:64*rh+64, zeros elsewhere.
            vv = [[p_blk.tile([128, 4, 512], FP16, tag=f"vv{rh}{b01}",
                              name=f"vv{rh}{b01}")
                   for b01 in range(2)] for rh in range(2)]
            if g < 4:
                for rh in range(2):
                    for b01 in range(2):
                        nc.vector.memset(
                            vv[rh][b01][64 * (1 - rh):64 * (1 - rh) + 64,
                                        :, :], 0)
            for rh in range(2):
                for b01 in range(2):
                    nc.sync.dma_start(
                        out=vv[rh][b01][64 * rh:64 * rh + 64, :, :],
                        in_=v_t[64 * b01:64 * b01 + 64, :, :])

            state[g] = dict(vv=vv, tok0=tok0, qt_t=qt_t, kt_t=kt_t,
                            dyn_t=dyn_t)

        def front_sm(g):
            """Scores + softmax + transpose for iteration g."""
            st = state[g]
            qt_t, kt_t, dyn_t = st["qt_t"], st["kt_t"], st["dyn_t"]

            # ---- scores + exp per board pair ----
            # sc[part = s + 64*par, col = 256*b01 + 64*jj + t]
            e_t = p_sc.tile([128, 4, 512], FP16, tag="e", name="e_t")
            for p in range(4):
                sc = ps_sc.tile([128, 512], F32, tag="sc", name="sc")
                nh = [0, 0]
                mms = {0: [], 1: []}
                for b01 in range(2):
                    bi8 = 2 * p + b01
                    for jj in range(4):
                        for par in range(2):
                            r0 = 64 * par
                            mms[par].append(nc.tensor.matmul(
                                sc[r0:r0 + 64,
                                   256 * b01 + 64 * jj:256 * b01 + 64 * jj + 64],
                                qt_t[r0:r0 + 64, jj, 64 * bi8:64 * bi8 + 64],
                                kt_t[r0:r0 + 64, jj, 64 * bi8:64 * bi8 + 64],
                                start=(nh[par] == 0), stop=(nh[par] == 7),
                                skip_group_check=True))
                            nh[par] += 1
                chain(mms[0])
                chain(mms[1])
                nc.scalar.activation(e_t[:, p, :], sc[:], AF.Exp, scale=1.0)

            # ---- mask multiply (fp16, DVE 2x) ----
            em_t = p_sc.tile([128, 4, 512], FP16, tag="em", name="em_t")
            for p in range(4):
                ev = e_t[:, p, :].rearrange("p (b c) -> p b c", b=2)
                emv = em_t[:, p, :].rearrange("p (b c) -> p b c", b=2)
                nc.vector.tensor_mul(
                    emv[:, :, 0:192], ev[:, :, 0:192],
                    stat_sb[:].rearrange("p (o c) -> p o c", o=1)
                    .broadcast_to((128, 2, 192)))
                nc.vector.tensor_mul(
                    emv[:, :, 192:256], ev[:, :, 192:256],
                    dyn_t[:, 128 * p:128 * p + 128]
                    .rearrange("p (b c) -> p b c", b=2))

            # ---- den / recip / normalize (fp16 for DVE 2x modes) ----
            den_t = p_sc.tile([128, 32], FP16, tag="den", name="den_t")
            rden_t = p_sc.tile([128, 32], FP16, tag="rden", name="rden_t")
            with nc.allow_low_precision(reason="softmax den fits fp16"):
                nc.vector.tensor_reduce(
                    den_t[:], em_t[:].rearrange("p m (h t) -> p (m h) t",
                                                t=64),
                    axis=mybir.AxisListType.X, op=ALU.add)
                nc.vector.reciprocal(rden_t[:], den_t[:])
            emn_t = p_sc.tile([128, 4, 512], FP16, tag="emn", name="emn_t")
            nc.gpsimd.tensor_mul(
                emn_t[:].rearrange("p m (h t) -> p (m h) t", t=64),
                em_t[:].rearrange("p m (h t) -> p (m h) t", t=64),
                rden_t[:].rearrange("p (n o) -> p n o", o=1)
                .broadcast_to((128, 32, 64)))

            state[g]["emn_t"] = emn_t

        def front_tr(g):
            """Transpose via DMA xbar (SBUF->SBUF, no PE/PSUM).  Emitted
            after back_op so the out-store is not queued behind it."""
            # at_all[a, blk=4p+qb, c] = emn[c, 128 blk + a]: each 128-col
            # block of emn transposed; rows a = 64*jj' + t, cols c = s+64par.
            at_all = p_at.tile([128, 16, 128], FP16, tag="at_all",
                               name="at_all")
            nc.sync.dma_start(out=at_all[:], in_=state[g]["emn_t"][:],
                              transpose=True)
            state[g]["at_all"] = at_all

        def back_av(g):
            """attn@v + evacuation for iteration g."""
            st = state[g]
            vv, at_all = st["vv"], st["at_all"]

            # ---- attn @ v ----
            aoh_t = p_ao.tile([128, 4, 512], F8, tag="aoh", name="aoh_t")
            aol_t = p_ao.tile([128, 4, 512], F8, tag="aol", name="aol_t")
            for c in range(4):
                rh, qb0 = c % 2, c // 2
                ao_ps = ps_ao.tile([128, 512], F32, tag="ao", name="ao_ps")
                nh = [0, 0]
                mms = {0: [], 1: []}
                for p in range(4):
                    for b01 in range(2):
                        qb = 2 * b01 + qb0
                        for par in range(2):
                            h = 2 * c + par
                            at_blk = at_all[:, 4 * p + qb,
                                            64 * par:64 * par + 64]
                            mms[par].append(nc.tensor.matmul(
                                ao_ps[64 * par:64 * par + 64,
                                      64 * (2 * p + b01):
                                      64 * (2 * p + b01) + 64],
                                vv[rh][b01][:, p, 64 * h:64 * h + 64],
                                at_blk,
                                start=(nh[par] == 0), stop=(nh[par] == 7),
                                skip_group_check=True))
                            nh[par] += 1
                chain(mms[0])
                chain(mms[1])
                nc.scalar.activation(aoh_t[:, c, :], ao_ps[:], AF.Identity,
                                     scale=1.0)
                with nc.allow_low_precision(reason="fp8 residual"):
                    nc.vector.tensor_sub(aol_t[:, c, :], ao_ps[:],
                                         aoh_t[:, c, :])
            st["aoh_t"], st["aol_t"] = aoh_t, aol_t

        def back_op(g):
            """Output projection + store for iteration g."""
            st = state.pop(g)
            aoh_t, aol_t, tok0 = st["aoh_t"], st["aol_t"], st["tok0"]

            # ---- output projection (fp8 DR comp-3) ----
            y_t = p_out.tile([128, 4, 512], FP16, tag="y", name="y_t")
            for mt in range(4):
                ps = ps_op.tile([128, 512], F32, tag="ao", name="op_ps")
                mms = []
                for kp in range(2):
                    asl = (slice(None), slice(2 * kp, 2 * kp + 2),
                           slice(128 * mt, 128 * mt + 128))
                    for wt, at_ in (("woh", aoh_t), ("wol", aoh_t),
                                    ("woh", aol_t)):
                        mms.append(nc.tensor.matmul(
                            ps[:], at_[asl],
                            w_sb[wt][:, 2 * kp:2 * kp + 2, :],
                            start=(len(mms) == 0), stop=(len(mms) == 5),
                            perf_mode=DR))
                chain(mms)
                nc.scalar.activation(y_t[:, mt, :], ps[:], AF.Identity,
                                     scale=1.0 / WS)
            nc.sync.dma_start(
                out=out[tok0:tok0 + 512, :]
                .rearrange("(m p) n -> p m n", p=128),
                in_=y_t[:])

        LAG = 3
        for g in range(n_g8 + LAG):
            if g < n_g8:
                front(g)
            if g >= LAG:
                back_av(g - LAG)
            if g < n_g8:
                front_sm(g)
            if g >= LAG:
                back_op(g - LAG)
            if g < n_g8:
                front_tr(g)

    nc.finalize()
    return nc


def prep_inputs(x, ray_mask, attack_mask, Wq, bq, Wk, bk, Wv, bv, Wo, bo,
                n_boards=BPC, core=None):
    """Host-side prep: fp8 hi/lo split, per-core slices, masks."""
    TOK = n_boards * S
    n_g8 = n_boards // 8
    eye = np.eye(64, dtype=bool)

    def split8(a):
        """a (f32) -> (hi, lo) fp8 pair with a ~= hi + lo."""
        hi = a.astype(f8)
        lo = (a - hi.astype(np.float32)).astype(f8)
        return hi, lo

    def wsplit(W, pre=1.0, perm_out=None, perm_in=None):
        """(W*pre*WS).T -> hi/lo fp8 chunked [4, 128, 512] (d_in chunks)."""
        Wt = (W * (pre * WS)).astype(np.float32)
        if perm_out is not None:
            Wt = Wt[perm_out]
        if perm_in is not None:
            Wt = Wt[:, perm_in]
        Wt = np.ascontiguousarray(Wt.T)         # [d_in, d_out]
        hi, lo = split8(Wt)
        return (np.ascontiguousarray(hi.reshape(4, 128, 512)),
                np.ascontiguousarray(lo.reshape(4, 128, 512)))

    wqh, wql = wsplit(Wq, pre=SCALE)
    wkh, wkl = wsplit(Wk)
    wvh, wvl = wsplit(Wv)
    woh, wol = wsplit(Wo)

    bqs = np.ascontiguousarray(
        (bq * SCALE).astype(np.float32).reshape(4, 128).T)  # [128, 4]

    stat = _static_masks()  # (6, 64, 64) bool
    # statm[part = s + 64*par, 64*jj + t] = static mask head 2*jj+par
    statm_h = np.zeros((128, 192), np.float32)
    for jj in range(3):
        for par in range(2):
            statm_h[64 * par:64 * par + 64, 64 * jj:64 * jj + 64] = \
                stat[2 * jj + par]
    statm_h = statm_h.astype(fp16)

    shared = dict(wqh=wqh, wql=wql, wkh=wkh, wkl=wkl, wvh=wvh, wvl=wvl,
                  woh=woh, wol=wol, bqs=bqs, statm=statm_h)

    cores = range(NCORES) if core is None else [core]
    in_maps = []
    for c in cores:
        xs = x[c * n_boards:(c + 1) * n_boards].reshape(TOK, 512)
        xt = np.ascontiguousarray(xs.T)         # [512, TOK] f32
        xh, xl = split8(xt)
        xh = np.ascontiguousarray(xh.reshape(4, 128, TOK))
        xl = np.ascontiguousarray(xl.reshape(4, 128, TOK))
        ray = (ray_mask[c * n_boards:(c + 1) * n_boards] | eye)
        atk = (attack_mask[c * n_boards:(c + 1) * n_boards] | eye)
        # dynm[g, part = s + 64*par, 128*p + 64*b01 + t]:
        #   par 0 -> head 6 = ray, par 1 -> head 7 = attack
        dynm = np.zeros((n_g8, 128, 512), np.float32)
        ray4 = ray.reshape(n_g8, 8, 64, 64)
        atk4 = atk.reshape(n_g8, 8, 64, 64)
        for p in range(4):
            for b01 in range(2):
                col = 128 * p + 64 * b01
                dynm[:, 0:64, col:col + 64] = ray4[:, 2 * p + b01]
                dynm[:, 64:128, col:col + 64] = atk4[:, 2 * p + b01]
        in_maps.append(dict(xh=xh, xl=xl, dynm=dynm.astype(fp16), **shared))
    return in_maps


def finish_output(raw, bo_eff, n_boards=BPC):
    """raw fp16 [TOK, 512] -> f32 [n_boards, S, D] with bo' added."""
    return raw.astype(np.float32).reshape(n_boards, S, D) + bo_eff


def bo_effective(Wo, bo, bv):
    return (bo.astype(np.float64)
            + bv.astype(np.float64) @ Wo.astype(np.float64).T
            ).astype(np.float32)


_NC_CACHE = {}


def kernel(**inputs):
    n_boards = BPC
    if "nc" not in _NC_CACHE:
        _NC_CACHE["nc"] = build_nc(n_boards)
    nc = _NC_CACHE["nc"]
    in_maps = prep_inputs(**inputs, n_boards=n_boards)
    res = run_bass_kernel_spmd(nc, in_maps, list(range(NCORES)))
    bo_eff = bo_effective(inputs["Wo"], inputs["bo"], inputs["bv"])
    outs = [finish_output(res.results[c]["out"], bo_eff, n_boards)
            for c in range(NCORES)]
    return np.concatenate(outs, axis=0)


if __name__ == "__main__":
    nc = build_nc()
    print("built ok")



# revision 84
# speedup vs baseline: 1.0005x; 1.0005x over previous
"""Trainium2 Bass kernel for chess-structured multi-head attention (8 cores).

Math (per board b of 2048, S=64 squares, D=512, H=8 heads, HD=64):
  q/k/v = x @ W{q,k,v}.T + b{q,k,v}
  scores_h = q_h k_h^T / 8, masked per head (6 static chess relations,
  ray, attack), softmax over targets, out = concat_h(attn_h v_h) @ Wo.T + bo

Sharding: pure data parallel, 256 boards per core; weights replicated.

Key tricks vs the v1 kernel (1.007ms):
  - All four projections run as fp8e4m3 DoubleRow matmuls with hi+lo error
    compensation: x = xh + xl (both fp8), W*64 = Wh + Wl; three DR terms
    xh*Wh + xh*Wl + xl*Wh at 0.5 cyc/row gives 0.75x the rows of bf16 at
    bf16-level accuracy (validated offline: rel err 2.5e-3 vs ref).
  - softmax shift-invariance: bk dropped entirely (q.bk const per row);
    SCALE and bq folded into Wq/bq on host; bv exits the device via
    bo' = bo + bv @ Wo.T added on host.
  - scores: per (board, jj) ONE matmul computes both heads 2jj/2jj+1 via a
    block-diagonal stationary q2 = [[q_even,0],[0,q_odd]] (s-parity-major
    free layout), out spans all 128 partitions -> 2048 rows/g8.
  - masks applied MULTIPLICATIVELY post-exp (fp16 0/1 masks, DVE 2x mode).
  - attn@v: per (at-block, col-parity) ONE matmul computes two heads via
    block-diag V tiles vb2 -> 2048 rows/g8.
  - intermediates fp16 (not bf16): same speed, 8x less quantization noise.
  - output written fp16; host upcasts and adds bo'.
"""

import os
import sys
from contextlib import ExitStack

import numpy as np
import ml_dtypes

for _p in ("/opt/trn_rl_repo", os.path.expanduser("~/.axon_site/_ro/trn_rl_repo")):
    if os.path.isdir(_p) and _p not in sys.path:
        sys.path.append(_p)

import concourse.bass as bass
import concourse.tile as tile
from concourse import bacc, mybir
from concourse.bass_utils import run_bass_kernel_spmd

F8 = mybir.dt.float8e4
BF16 = mybir.dt.bfloat16
FP16 = mybir.dt.float16
F32 = mybir.dt.float32
f8 = ml_dtypes.float8_e4m3
bf16 = ml_dtypes.bfloat16
fp16 = np.float16

B, S, D, H, HD = 2048, 64, 512, 8, 64
NCORES = 8
BPC = B // NCORES
SCALE = float(1.0 / np.sqrt(HD))
WS = 64.0                       # weight pre-scale into fp8 normal range
DR = mybir.MatmulPerfMode.DoubleRow



def _static_masks():
    sq = np.arange(64)
    r = sq // 8
    f = sq % 8
    ri, rj = r[:, None], r[None, :]
    fi, fj = f[:, None], f[None, :]
    dr = np.abs(ri - rj)
    df = np.abs(fi - fj)
    eye = np.eye(64, dtype=bool)
    file_m = fi == fj
    rank_m = ri == rj
    diag_m = (ri - fi) == (rj - fj)
    adiag_m = (ri + fi) == (rj + fj)
    knight_m = (((dr == 2) & (df == 1)) | ((dr == 1) & (df == 2))) | eye
    king_m = (dr <= 1) & (df <= 1)
    return np.stack([file_m, rank_m, diag_m, adiag_m, knight_m, king_m])


def build_nc(n_boards=BPC):
    """Single-core Bass program (SPMD across 8 cores)."""
    assert n_boards % 8 == 0
    n_g8 = n_boards // 8
    TOK = n_boards * S

    nc = bacc.Bacc(None)

    # DRAM I/O.  x hi/lo fp8 chunks: [4 kchunks, 128, TOK]
    xh_d = nc.dram_tensor("xh", [4, 128, TOK], F8, kind="ExternalInput")
    xl_d = nc.dram_tensor("xl", [4, 128, TOK], F8, kind="ExternalInput")
    dynm_d = nc.dram_tensor("dynm", [n_g8, 128, 512], FP16,
                            kind="ExternalInput")
    w_d = {}
    for name in ("wqh", "wql", "wkh", "wkl", "wvh", "wvl", "woh", "wol"):
        w_d[name] = nc.dram_tensor(name, [4, 128, 512], F8,
                                   kind="ExternalInput")
    bq_d = nc.dram_tensor("bqs", [128, 4], F32, kind="ExternalInput")
    statm_d = nc.dram_tensor("statm", [128, 192], FP16, kind="ExternalInput")
    out = nc.dram_tensor("out", [TOK, 512], FP16, kind="ExternalOutput")

    AF = mybir.ActivationFunctionType
    ALU = mybir.AluOpType

    def chain(insts):
        for b_ in insts[1:]:
            tile.add_dep_helper(b_.ins, insts[0].ins, sync=False,
                                reason="psum group start-first")
        for a in insts[1:-1]:
            tile.add_dep_helper(insts[-1].ins, a.ins, sync=False,
                                reason="psum group stop-last")

    with tile.TileContext(nc) as tc, ExitStack() as ctx:
        const = ctx.enter_context(tc.tile_pool(name="const", bufs=1))
        p_x = ctx.enter_context(tc.tile_pool(name="x", bufs=3))
        p_qkv = ctx.enter_context(tc.tile_pool(name="qkv", bufs=3))
        p_blk = ctx.enter_context(tc.tile_pool(name="blk", bufs=4))
        p_sc = ctx.enter_context(tc.tile_pool(name="sc", bufs=4))
        p_at = ctx.enter_context(tc.tile_pool(name="at", bufs=4))
        p_ao = ctx.enter_context(tc.tile_pool(name="ao", bufs=3))
        p_out = ctx.enter_context(tc.tile_pool(name="out", bufs=3))
        # PSUM: 8 banks total.  proj 3 + scores 2 + (ao|oproj shared) 3
        ps_proj = ctx.enter_context(
            tc.tile_pool(name="ps_proj", bufs=3, space="PSUM"))
        ps_sc = ctx.enter_context(
            tc.tile_pool(name="ps_sc", bufs=2, space="PSUM"))
        ps_ao = ctx.enter_context(
            tc.tile_pool(name="ps_ao", bufs=3, space="PSUM"))
        ps_op = ps_ao

        # ---- constants (Q weights first, then iter-0/1 x prefetch,
        #      so the first projection can start ~6us earlier) ----
        w_sb = {}
        preload = {}
        # interleave: first Q matmul needs only wqh+xh0; wql+xl0 next
        w = const.tile([128, 4, 512], F8, tag="wqh", name="w")
        nc.sync.dma_start(out=w[:], in_=w_d["wqh"].rearrange("k p n -> p k n"))
        w_sb["wqh"] = w
        pxh = p_x.tile([128, 4, 512], F8, tag="xh", name="xh_t")
        nc.sync.dma_start(out=pxh[:], in_=xh_d[:, :, 0:512]
                          .rearrange("k p n -> p k n"))
        w = const.tile([128, 4, 512], F8, tag="wql", name="w")
        nc.sync.dma_start(out=w[:], in_=w_d["wql"].rearrange("k p n -> p k n"))
        w_sb["wql"] = w
        pxl = p_x.tile([128, 4, 512], F8, tag="xl", name="xl_t")
        nc.sync.dma_start(out=pxl[:], in_=xl_d[:, :, 0:512]
                          .rearrange("k p n -> p k n"))
        pdyn = p_sc.tile([128, 512], FP16, tag="dyn", name="dyn_t")
        nc.sync.dma_start(out=pdyn[:], in_=dynm_d[0, :, :])
        preload[0] = (pxh, pxl, pdyn)
        bq_sb = const.tile([128, 4], F32, tag="bqs", name="bq_sb")
        nc.sync.dma_start(out=bq_sb[:], in_=bq_d[:])
        # dummy exp: hoists the one-time ACT table load into the DMA wait
        warm_t = const.tile([128, 1], F32, tag="warm", name="warm_t")
        nc.scalar.activation(warm_t[:], bq_sb[:, 0:1], AF.Exp, scale=0.0)
        for name, t in w_d.items():
            if name in w_sb:
                continue
            w = const.tile([128, 4, 512], F8, tag=name, name="w")
            nc.sync.dma_start(out=w[:], in_=t.rearrange("k p n -> p k n"))
            w_sb[name] = w
        stat_sb = const.tile([128, 192], FP16, tag="statm")
        nc.sync.dma_start(out=stat_sb[:], in_=statm_d[:])

        def proj_dr(psout, wh, wl, xh_t, xl_t, col0, terms=3):
            """Compensated fp8 DR projection into psout [128, 512].

            terms=3: xh*Wh + xh*Wl + xl*Wh;  terms=2 drops xl*Wh (used for
            Q only: softmax tolerates ~1e-2 there, halves nothing else).
            lhsT = w[:, kp-pair, col0:col0+128]; rhs = x[:, kp-pair, :].
            """
            half = terms == 2.5
            tl = ((wh, xh_t), (wl, xh_t), (wh, xl_t))[:2 if half else terms]
            n = 2 * len(tl) + (1 if half else 0)
            mms = []
            for kp in range(2):
                sl = (slice(None), slice(2 * kp, 2 * kp + 2),
                      slice(col0, col0 + 128))
                for wt, xt_ in tl:
                    mms.append(nc.tensor.matmul(
                        psout[:], wt[sl], xt_[:, 2 * kp:2 * kp + 2, :],
                        start=(len(mms) == 0), stop=(len(mms) == n - 1),
                        perf_mode=DR))
                if half and kp == 0:
                    mms.append(nc.tensor.matmul(
                        psout[:], wh[sl], xl_t[:, 0:2, :],
                        start=False, stop=False, perf_mode=DR))
            chain(mms)

        state = {}

        def front(g):
            """Loads, Q/K/V projections, scores, softmax -> state[g]."""
            tok0 = g * 512

            # ---- loads: prefetch iteration g+1, consume prefetch g ----
            if g + 1 < n_g8:
                t1 = (g + 1) * 512
                nxh = p_x.tile([128, 4, 512], F8, tag="xh", name="xh_t")
                nxl = p_x.tile([128, 4, 512], F8, tag="xl", name="xl_t")
                nc.sync.dma_start(out=nxh[:],
                                  in_=xh_d[:, :, t1:t1 + 512]
                                  .rearrange("k p n -> p k n"))
                nc.sync.dma_start(out=nxl[:],
                                  in_=xl_d[:, :, t1:t1 + 512]
                                  .rearrange("k p n -> p k n"))
                ndyn = p_sc.tile([128, 512], FP16, tag="dyn", name="dyn_t")
                nc.sync.dma_start(out=ndyn[:], in_=dynm_d[g + 1, :, :])
                preload[g + 1] = (nxh, nxl, ndyn)
            xh_t, xl_t, dyn_t = preload.pop(g)

            # ---- Q/K projections (transposed: d on partitions) ----
            qt_t = p_qkv.tile([128, 4, 512], FP16, tag="qt", name="qt_t")
            kt_t = p_qkv.tile([128, 4, 512], FP16, tag="kt", name="kt_t")
            for jj in range(4):
                ps = ps_proj.tile([128, 512], F32, tag="proj")
                proj_dr(ps, w_sb["wqh"], w_sb["wql"], xh_t, xl_t,
                        128 * jj, terms=2)
                nc.scalar.activation(qt_t[:, jj, :], ps[:], AF.Identity,
                                     bias=bq_sb[:, jj:jj + 1], scale=1.0 / WS)
            for jj in range(4):
                ps = ps_proj.tile([128, 512], F32, tag="proj")
                proj_dr(ps, w_sb["wkh"], w_sb["wkl"], xh_t, xl_t,
                        128 * jj, terms=2.5)
                nc.scalar.activation(kt_t[:, jj, :], ps[:], AF.Identity,
                                     scale=1.0 / WS)

            # ---- V projection (natural: tokens on partitions) ----
            v_t = p_qkv.tile([128, 4, 512], FP16, tag="v", name="v_t")
            for mt in range(4):
                ps = ps_proj.tile([128, 512], F32, tag="proj")
                mms = []
                for kp in range(2):
                    xsl = (slice(None), slice(2 * kp, 2 * kp + 2),
                           slice(128 * mt, 128 * mt + 128))
                    for wt, xt_ in (("wvh", xh_t), ("wvl", xh_t),
                                    ("wvh", xl_t)):
                        mms.append(nc.tensor.matmul(
                            ps[:], xt_[xsl],
                            w_sb[wt][:, 2 * kp:2 * kp + 2, :],
                            start=(len(mms) == 0), stop=(len(mms) == 5),
                            perf_mode=DR))
                chain(mms)
                nc.vector.tensor_scalar_mul(v_t[:, mt, :], ps[:], 1.0 / WS)

            # vv: zero-padded V tiles: vv[rh][b01] has board-b01 tokens
            # live on rows 64*rh:64*rh+64, zeros elsewhere.
            vv = [[p_blk.tile([128, 4, 512], FP16, tag=f"vv{rh}{b01}",
                              name=f"vv{rh}{b01}")
                   for b01 in range(2)] for rh in range(2)]
            if g < 4:
                for rh in range(2):
                    for b01 in range(2):
                        nc.vector.memset(
                            vv[rh][b01][64 * (1 - rh):64 * (1 - rh) + 64,
                                        :, :], 0)
            for rh in range(2):
                for b01 in range(2):
                    nc.sync.dma_start(
                        out=vv[rh][b01][64 * rh:64 * rh + 64, :, :],
                        in_=v_t[64 * b01:64 * b01 + 64, :, :])

            state[g] = dict(vv=vv, tok0=tok0, qt_t=qt_t, kt_t=kt_t,
                            dyn_t=dyn_t)

        def front_sm(g):
            """Scores + softmax + transpose for iteration g."""
            st = state[g]
            qt_t, kt_t, dyn_t = st["qt_t"], st["kt_t"], st["dyn_t"]

            # ---- scores + exp per board pair ----
            # sc[part = s + 64*par, col = 256*b01 + 64*jj + t]
            e_t = p_sc.tile([128, 4, 512], FP16, tag="e", name="e_t")
            for p in range(4):
                sc = ps_sc.tile([128, 512], F32, tag="sc", name="sc")
                nh = [0, 0]
                mms = {0: [], 1: []}
                for b01 in range(2):
                    bi8 = 2 * p + b01
                    for jj in range(4):
                        for par in range(2):
                            r0 = 64 * par
                            mms[par].append(nc.tensor.matmul(
                                sc[r0:r0 + 64,
                                   256 * b01 + 64 * jj:256 * b01 + 64 * jj + 64],
                                qt_t[r0:r0 + 64, jj, 64 * bi8:64 * bi8 + 64],
                                kt_t[r0:r0 + 64, jj, 64 * bi8:64 * bi8 + 64],
                                start=(nh[par] == 0), stop=(nh[par] == 7),
                                skip_group_check=True))
                            nh[par] += 1
                chain(mms[0])
                chain(mms[1])
                nc.scalar.activation(e_t[:, p, :], sc[:], AF.Exp, scale=1.0)

            # ---- mask multiply (fp16, DVE 2x) ----
            em_t = p_sc.tile([128, 4, 512], FP16, tag="em", name="em_t")
            for p in range(4):
                ev = e_t[:, p, :].rearrange("p (b c) -> p b c", b=2)
                emv = em_t[:, p, :].rearrange("p (b c) -> p b c", b=2)
                nc.vector.tensor_mul(
                    emv[:, :, 0:192], ev[:, :, 0:192],
                    stat_sb[:].rearrange("p (o c) -> p o c", o=1)
                    .broadcast_to((128, 2, 192)))
                nc.vector.tensor_mul(
                    emv[:, :, 192:256], ev[:, :, 192:256],
                    dyn_t[:, 128 * p:128 * p + 128]
                    .rearrange("p (b c) -> p b c", b=2))

            # ---- den / recip / normalize (fp16 for DVE 2x modes) ----
            den_t = p_sc.tile([128, 32], FP16, tag="den", name="den_t")
            rden_t = p_sc.tile([128, 32], FP16, tag="rden", name="rden_t")
            with nc.allow_low_precision(reason="softmax den fits fp16"):
                nc.vector.tensor_reduce(
                    den_t[:], em_t[:].rearrange("p m (h t) -> p (m h) t",
                                                t=64),
                    axis=mybir.AxisListType.X, op=ALU.add)
                nc.vector.reciprocal(rden_t[:], den_t[:])
            emn_t = p_sc.tile([128, 4, 512], FP16, tag="emn", name="emn_t")
            nc.gpsimd.tensor_mul(
                emn_t[:].rearrange("p m (h t) -> p (m h) t", t=64),
                em_t[:].rearrange("p m (h t) -> p (m h) t", t=64),
                rden_t[:].rearrange("p (n o) -> p n o", o=1)
                .broadcast_to((128, 32, 64)))

            state[g]["emn_t"] = emn_t

        def front_tr(g):
            """Transpose via DMA xbar (SBUF->SBUF, no PE/PSUM).  Emitted
            after back_op so the out-store is not queued behind it."""
            # at_all[a, blk=4p+qb, c] = emn[c, 128 blk + a]: each 128-col
            # block of emn transposed; rows a = 64*jj' + t, cols c = s+64par.
            at_all = p_at.tile([128, 16, 128], FP16, tag="at_all",
                               name="at_all")
            nc.sync.dma_start(out=at_all[:], in_=state[g]["emn_t"][:],
                              transpose=True)
            state[g]["at_all"] = at_all

        def back_av(g):
            """attn@v + evacuation for iteration g."""
            st = state[g]
            vv, at_all = st["vv"], st["at_all"]

            # ---- attn @ v ----
            aoh_t = p_ao.tile([128, 4, 512], F8, tag="aoh", name="aoh_t")
            aol_t = p_ao.tile([128, 4, 512], F8, tag="aol", name="aol_t")
            for c in range(4):
                rh, qb0 = c % 2, c // 2
                ao_ps = ps_ao.tile([128, 512], F32, tag="ao", name="ao_ps")
                nh = [0, 0]
                mms = {0: [], 1: []}
                for p in range(4):
                    for b01 in range(2):
                        qb = 2 * b01 + qb0
                        for par in range(2):
                            h = 2 * c + par
                            at_blk = at_all[:, 4 * p + qb,
                                            64 * par:64 * par + 64]
                            mms[par].append(nc.tensor.matmul(
                                ao_ps[64 * par:64 * par + 64,
                                      64 * (2 * p + b01):
                                      64 * (2 * p + b01) + 64],
                                vv[rh][b01][:, p, 64 * h:64 * h + 64],
                                at_blk,
                                start=(nh[par] == 0), stop=(nh[par] == 7),
                                skip_group_check=True))
                            nh[par] += 1
                chain(mms[0])
                chain(mms[1])
                nc.scalar.activation(aoh_t[:, c, :], ao_ps[:], AF.Identity,
                                     scale=1.0)
                with nc.allow_low_precision(reason="fp8 residual"):
                    nc.vector.tensor_sub(aol_t[:, c, :], ao_ps[:],
                                         aoh_t[:, c, :])
            st["aoh_t"], st["aol_t"] = aoh_t, aol_t

        def back_op(g):
            """Output projection + store for iteration g."""
            st = state.pop(g)
            aoh_t, aol_t, tok0 = st["aoh_t"], st["aol_t"], st["tok0"]

            # ---- output projection (fp8 DR comp-3) ----
            y_t = p_out.tile([128, 4, 512], FP16, tag="y", name="y_t")
            for mt in range(4):
                ps = ps_op.tile([128, 512], F32, tag="ao", name="op_ps")
                mms = []
                for kp in range(2):
                    asl = (slice(None), slice(2 * kp, 2 * kp + 2),
                           slice(128 * mt, 128 * mt + 128))
                    for wt, at_ in (("woh", aoh_t), ("wol", aoh_t),
                                    ("woh", aol_t)):
                        mms.append(nc.tensor.matmul(
                            ps[:], at_[asl],
                            w_sb[wt][:, 2 * kp:2 * kp + 2, :],
                            start=(len(mms) == 0), stop=(len(mms) == 5),
                            perf_mode=DR))
                chain(mms)
                nc.scalar.activation(y_t[:, mt, :], ps[:], AF.Identity,
                                     scale=1.0 / WS)
            nc.sync.dma_start(
                out=out[tok0:tok0 + 512, :]
                .rearrange("(m p) n -> p m n", p=128),
                in_=y_t[:])

        LAG = 3
        for g in range(n_g8 + LAG):
            if g < n_g8:
                front(g)
            if g >= LAG:
                back_av(g - LAG)
            if g < n_g8:
                front_sm(g)
            if g >= LAG:
                back_op(g - LAG)
            if g < n_g8:
                front_tr(g)

    nc.finalize()
    return nc


def prep_inputs(x, ray_mask, attack_mask, Wq, bq, Wk, bk, Wv, bv, Wo, bo,
                n_boards=BPC, core=None):
    """Host-side prep: fp8 hi/lo split, per-core slices, masks."""
    TOK = n_boards * S
    n_g8 = n_boards // 8
    eye = np.eye(64, dtype=bool)

    def split8(a):
        """a (f32) -> (hi, lo) fp8 pair with a ~= hi + lo."""
        hi = a.astype(f8)
        lo = (a - hi.astype(np.float32)).astype(f8)
        return hi, lo

    def wsplit(W, pre=1.0, perm_out=None, perm_in=None):
        """(W*pre*WS).T -> hi/lo fp8 chunked [4, 128, 512] (d_in chunks)."""
        Wt = (W * (pre * WS)).astype(np.float32)
        if perm_out is not None:
            Wt = Wt[perm_out]
        if perm_in is not None:
            Wt = Wt[:, perm_in]
        Wt = np.ascontiguousarray(Wt.T)         # [d_in, d_out]
        hi, lo = split8(Wt)
        return (np.ascontiguousarray(hi.reshape(4, 128, 512)),
                np.ascontiguousarray(lo.reshape(4, 128, 512)))

    wqh, wql = wsplit(Wq, pre=SCALE)
    wkh, wkl = wsplit(Wk)
    wvh, wvl = wsplit(Wv)
    woh, wol = wsplit(Wo)

    bqs = np.ascontiguousarray(
        (bq * SCALE).astype(np.float32).reshape(4, 128).T)  # [128, 4]

    stat = _static_masks()  # (6, 64, 64) bool
    # statm[part = s + 64*par, 64*jj + t] = static mask head 2*jj+par
    statm_h = np.zeros((128, 192), np.float32)
    for jj in range(3):
        for par in range(2):
            statm_h[64 * par:64 * par + 64, 64 * jj:64 * jj + 64] = \
                stat[2 * jj + par]
    statm_h = statm_h.astype(fp16)

    shared = dict(wqh=wqh, wql=wql, wkh=wkh, wkl=wkl, wvh=wvh, wvl=wvl,
                  woh=woh, wol=wol, bqs=bqs, statm=statm_h)

    cores = range(NCORES) if core is None else [core]
    in_maps = []
    for c in cores:
        xs = x[c * n_boards:(c + 1) * n_boards].reshape(TOK, 512)
        xt = np.ascontiguousarray(xs.T)         # [512, TOK] f32
        xh, xl = split8(xt)
        xh = np.ascontiguousarray(xh.reshape(4, 128, TOK))
        xl = np.ascontiguousarray(xl.reshape(4, 128, TOK))
        ray = (ray_mask[c * n_boards:(c + 1) * n_boards] | eye)
        atk = (attack_mask[c * n_boards:(c + 1) * n_boards] | eye)
        # dynm[g, part = s + 64*par, 128*p + 64*b01 + t]:
        #   par 0 -> head 6 = ray, par 1 -> head 7 = attack
        dynm = np.zeros((n_g8, 128, 512), np.float32)
        ray4 = ray.reshape(n_g8, 8, 64, 64)
        atk4 = atk.reshape(n_g8, 8, 64, 64)
        for p in range(4):
            for b01 in range(2):
                col = 128 * p + 64 * b01
                dynm[:, 0:64, col:col + 64] = ray4[:, 2 * p + b01]
                dynm[:, 64:128, col:col + 64] = atk4[:, 2 * p + b01]
        in_maps.append(dict(xh=xh, xl=xl, dynm=dynm.astype(fp16), **shared))
    return in_maps


def finish_output(raw, bo_eff, n_boards=BPC):
    """raw fp16 [TOK, 512] -> f32 [n_boards, S, D] with bo' added."""
    return raw.astype(np.float32).reshape(n_boards, S, D) + bo_eff


def bo_effective(Wo, bo, bv):
    return (bo.astype(np.float64)
            + bv.astype(np.float64) @ Wo.astype(np.float64).T
            ).astype(np.float32)


_NC_CACHE = {}


def kernel(**inputs):
    n_boards = BPC
    if "nc" not in _NC_CACHE:
        _NC_CACHE["nc"] = build_nc(n_boards)
    nc = _NC_CACHE["nc"]
    in_maps = prep_inputs(**inputs, n_boards=n_boards)
    res = run_bass_kernel_spmd(nc, in_maps, list(range(NCORES)))
    bo_eff = bo_effective(inputs["Wo"], inputs["bo"], inputs["bv"])
    outs = [finish_output(res.results[c]["out"], bo_eff, n_boards)
            for c in range(NCORES)]
    return np.concatenate(outs, axis=0)


if __name__ == "__main__":
    nc = build_nc()
    print("built ok")

